# revision 43
# baseline (speedup 1.0000x reference)
"""GQA sliding-window attention (B=1, T=2048, C=2048, 32 Q / 8 KV heads,
head_dim=64, window=512, 16 global tokens) on 8 Trainium2 NeuronCores.

Sharding: tensor-parallel over heads — core c owns KV head c and Q heads
4c..4c+3.  Everything on-device runs transposed ([feature, token] layout).

v2 layout vs baseline:
  * fully fused per-512-token-chunk pipeline: QKV+RoPE(i) -> WO(i-2) ->
    attention(i) -> AllGather(i) trigger -> prefetch WO inputs (i-1).
    Keeps the PE dense (HAM stays warm) and hides each chunk's AllGather
    behind the next chunk's compute.
  * bf16 for everything DMA'd or collective'd: x, wq/wkv/wo, probs, V,
    y, the AllGather payload and the WO matmul inputs.  q/k stay f32r
    in SBUF (free: same PE rate at moving>=256, no DMA cost).
  * softmax denominator reciprocal via one custom-DVE op
    (reciprocal_approx_fast) instead of scalar Ln+Exp — no ACT table
    thrash.
"""

import sys
sys.path.insert(0, "/opt/trn_rl_repo")

import numpy as np

import concourse.bass as bass
import concourse.mybir as mybir
from concourse import bacc
from concourse.tile import TileContext
from concourse.masks import make_identity

f32 = mybir.dt.float32
f32r = mybir.dt.float32r
bf16 = mybir.dt.bfloat16
AF = mybir.ActivationFunctionType

N_CORES = 8
T = 2048
C = 2048
HD = 64
NH_LOC = 4            # query heads per core
QD = NH_LOC * HD      # 256 per-core query dims
NB = T // 128         # 16 token blocks
NSB = T // 512        # 4 superblocks
N_GLOBAL = 16
SCALE = 0.125         # 1/sqrt(64)

_CACHE = {}


def _build():
    nc = bacc.Bacc(num_devices=N_CORES)

    xT = nc.declare_dram_parameter("xT", [C, T], bf16, isOutput=False)
    wqT = nc.declare_dram_parameter("wqT", [C, QD], bf16, isOutput=False)
    wkvT = nc.declare_dram_parameter("wkvT", [C, 128], bf16, isOutput=False)
    woT = nc.declare_dram_parameter("woT", [C, QD], bf16, isOutput=False)
    cs1 = nc.declare_dram_parameter("cs128", [128, T], f32, isOutput=False)
    sn1 = nc.declare_dram_parameter("sn128", [128, T], f32, isOutput=False)
    outT = nc.declare_dram_parameter("outT", [QD, T], f32, isOutput=True)

    with TileContext(nc) as tc:
        with tc.tile_pool(name="persist", bufs=1) as pp, \
             tc.tile_pool(name="psAll", bufs=1, space="PSUM") as psA, \
             tc.tile_pool(name="pdram", bufs=1, space="DRAM") as pdr:
            # ---- persistent state ----
            qTr01 = pp.tile([128, T], bf16)   # rotated q, heads 0,1
            qTr23 = pp.tile([128, T], bf16)   # rotated q, heads 2,3
            kTr2 = pp.tile([128, T], bf16)    # rotated k duplicated on halves
            vgs = [pp.tile([128, HD + 1], bf16, name=f"vg{j}", tag=f"vg{j}")
                   for j in range(NB)]
            csb = pp.tile([128, T], f32)
            snb = pp.tile([128, T], f32)
            identb = pp.tile([128, 128], bf16)
            m_diag = pp.tile([128, 128], f32)
            m_edge = pp.tile([128, 128], f32)
            mg16 = pp.tile([128, 512], f32)
            madd_diag = pp.tile([128, 128], bf16)
            madd_edge = pp.tile([128, 128], bf16)
            madd_g16 = pp.tile([128, 512], bf16)
            ones1 = pp.tile([128, 1], f32)
            wq_sb = [pp.tile([128, QD], bf16, name=f"wq{k}", tag=f"wq{k}")
                     for k in range(16)]
            wkv_sb = [pp.tile([128, 128], bf16, name=f"wkv{k}", tag=f"wkv{k}")
                      for k in range(16)]
            wo_sb = [pp.tile([128, QD], bf16, name=f"wo{k}", tag=f"wo{k}")
                     for k in range(16)]

            # additive score masks, folded into the scores psum via one
            # extra PE matmul (identb.T @ madd_*): 0 keeps, -1e5 kills
            # (exp underflows to exactly 0).  scoresT layout:
            # partition=key, free=query; diag keeps p <= f (causal),
            # edge keeps p > f (window lower edge), g16 kills window
            # copies of global keys (partitions < 16) in k-block 0.
            NEG = -1.0e5
            ib_f = pp.tile([128, 128], f32)
            make_identity(nc, ib_f[:])
            nc.gpsimd.tensor_copy(identb[:], ib_f[:])
            nc.gpsimd.memset(m_diag[:], 0.0)
            nc.gpsimd.affine_select(out=m_diag[:], in_=m_diag[:],
                                    compare_op=mybir.AluOpType.is_ge,
                                    fill=NEG,
                                    base=0, pattern=[[1, 128]],
                                    channel_multiplier=-1)
            nc.gpsimd.memset(m_edge[:], 0.0)
            nc.gpsimd.affine_select(out=m_edge[:], in_=m_edge[:],
                                    compare_op=mybir.AluOpType.is_ge,
                                    fill=NEG,
                                    base=-1, pattern=[[-1, 128]],
                                    channel_multiplier=1)
            nc.gpsimd.memset(mg16[:], 0.0)
            nc.gpsimd.affine_select(out=mg16[:], in_=mg16[:],
                                    compare_op=mybir.AluOpType.is_ge,
                                    fill=NEG,
                                    base=-N_GLOBAL, pattern=[[0, 512]],
                                    channel_multiplier=1)
            nc.vector.memset(ones1[:], 1.0)
            nc.gpsimd.tensor_copy(madd_diag[:], m_diag[:])
            nc.gpsimd.tensor_copy(madd_edge[:], m_edge[:])
            nc.gpsimd.tensor_copy(madd_g16[:], mg16[:])

            # psum banks (8):
            #   q01/q23/kv: QKV accum, then score-block rotation slots
            #   swd: 4th score rotation slot
            #   yq (bufs=2): PV accum ping-pong across heads
            #   ms: vT transposes + global scores + 2nd WO bank
            #   wp: WO accum
            def ps_tile(shape, tag, bufs=1):
                return psA.tile(shape, f32, tag=tag, bufs=bufs, name=tag,
                                padded_shape=[128, 512])

            agi = [pdr.tile([QD, 512], bf16, name=f"agi{Q}", tag=f"agi{Q}")
                   for Q in range(NSB)]
            ago = [pdr.tile([C, 512], bf16, name=f"ago{Q}", tag=f"ago{Q}",
                            addr_space="Shared") for Q in range(NSB)]

            with tc.tile_pool(name="pbc", bufs=1) as pbc:

                def rope(dst, psrc, rows, col0, pa):
                    """dst[0:rows, col0:+512] = rope(psrc); head dims are
                    pre-permuted to rotate-half order (evens then odds).
                    The partner-swap reads psum at a 32-partition offset
                    directly — no staging copy."""
                    t1 = pa.tile([rows, 512], f32, tag="t1", bufs=2)
                    t2 = pa.tile([rows, 512], f32, tag="t2", bufs=2)
                    nc.vector.tensor_mul(t1[:], psrc,
                                         csb[0:rows, col0:col0 + 512])
                    for b in range(rows // 32):
                        s = b ^ 1
                        nc.vector.tensor_mul(
                            t2[32 * b:32 * b + 32, :],
                            psrc[32 * s:32 * s + 32, :],
                            snb[32 * b:32 * b + 32, col0:col0 + 512])
                    nc.vector.tensor_add(dst[0:rows, col0:col0 + 512],
                                         t1[:], t2[:])

                def qkv_chunk(tcc, pa):
                    c0 = 512 * tcc
                    q01 = ps_tile([128, 512], "q01")
                    q23 = ps_tile([128, 512], "q23")
                    kv = ps_tile([128, 512], "kv")
                    # one batched 2MB DMA for the whole chunk of x
                    # (k-tile kt lands at columns 512*kt)
                    xtb = pa.tile([128, 16 * 512], bf16, tag="xt", bufs=2)
                    nc.sync.dma_start(
                        out=xtb[:].rearrange("p (g c) -> p g c", g=16),
                        in_=xT[:, c0:c0 + 512].rearrange(
                            "(g p) c -> p g c", g=16))
                    if tcc == 0:
                        for kt in range(16):
                            nc.sync.dma_start(
                                out=wq_sb[kt][:],
                                in_=wqT[128 * kt:128 * (kt + 1), :])
                            nc.sync.dma_start(
                                out=wkv_sb[kt][:],
                                in_=wkvT[128 * kt:128 * (kt + 1), :])
                        nc.sync.dma_start(out=csb[:], in_=cs1[:])
                        nc.sync.dma_start(out=snb[:], in_=sn1[:])
                    if tcc == 0:
                        # wo weights on the gpsimd queue, emitted before
                        # AG(0)'s trigger so they never queue behind a
                        # collective wait; sync stays free for xt
                        for kt in range(16):
                            nc.gpsimd.dma_start(
                                out=wo_sb[kt][:],
                                in_=woT[128 * kt:128 * (kt + 1), :])
                    for kt in range(16):
                        xt = xtb[:, 512 * kt:512 * (kt + 1)]
                        st, sp = kt == 0, kt == 15
                        nc.tensor.matmul(q01[0:128, :],
                                         wq_sb[kt][:, 0:128],
                                         xt, start=st, stop=sp)
                        nc.tensor.matmul(q23[0:128, :],
                                         wq_sb[kt][:, 128:256],
                                         xt, start=st, stop=sp)
                        nc.tensor.matmul(kv[0:128, :], wkv_sb[kt][:],
                                         xt, start=st, stop=sp)
                    rope(kTr2, kv[0:64, :], 64, c0, pa)
                    for b in range(2):
                        nc.vector.tensor_copy(
                            kTr2[64 + 32 * b:96 + 32 * b, c0:c0 + 512],
                            kTr2[32 * b:32 * (b + 1), c0:c0 + 512])
                    rope(qTr01, q01[0:128, :], 128, c0, pa)
                    rope(qTr23, q23[0:128, :], 128, c0, pa)
                    vt = pa.tile([64, 512], bf16, tag="vt", bufs=2)
                    # scalar copy: frees the vgs transpose DMAs from the
                    # DVE rope queue (scalar is idle until the first exp)
                    nc.scalar.copy(vt[:], kv[64:128, :])
                    # vgs via the DMA xbar transpose engine: keeps the PE
                    # and DVE queues out of the v-transpose dependency
                    # (ones columns pre-written once at startup)
                    for jj in range(4):
                        j = 4 * tcc + jj
                        nc.sync.dma_start_transpose(
                            out=vgs[j][:, 0:HD],
                            in_=vt[:, 128 * jj:128 * (jj + 1)])

                def attn_pair(p, Q, den, yus):
                    """heads (2p, 2p+1): head 2p on array rows 0:64, head
                    2p+1 on rows 64:128 (kTr2 duplicated on both halves
                    for this) — paired score matmuls target different
                    row groups and execute concurrently on the PE."""
                    qt = qTr01 if p == 0 else qTr23
                    c0 = 512 * Q
                    yqs = [ps_tile([HD + 1, 512], "yq", bufs=2)
                           for _ in range(2)]
                    pgs = []
                    for hh in range(2):
                        qb = 64 * hh
                        sg = ps_tile([N_GLOBAL, 512], "ms")
                        nc.tensor.matmul(sg[0:N_GLOBAL, :],
                                         kTr2[qb:qb + 64, 0:N_GLOBAL],
                                         qt[qb:qb + 64, c0:c0 + 512],
                                         start=True, stop=True)
                        pg = pbc.tile([N_GLOBAL, 512], bf16,
                                      tag=f"pg{hh}", bufs=1)
                        nc.scalar.activation(pg[:], sg[0:N_GLOBAL, :],
                                             AF.Exp, scale=SCALE)
                        pgs.append(pg)
                    js = list(range(max(0, 4 * Q - 4), 4 * Q + 4))
                    DEPTH = 2
                    pts = {}

                    def scores2(j):
                        qlo = max(4 * Q, j)
                        qhi = min(4 * Q + 3, j + 4)
                        s = 128 * (qhi - qlo + 1)
                        sws = []
                        for hh in range(2):
                            qb = 64 * hh
                            sw = ps_tile([128, s],
                                         ("q01", "q23", "kv", "swd")[
                                             (2 * j + hh) % 4])
                            nc.tensor.matmul(
                                sw[0:128, 0:s],
                                kTr2[qb:qb + 64, 128 * j:128 * (j + 1)],
                                qt[qb:qb + 64, 128 * qlo:128 * qlo + s],
                                start=True, stop=False)
                            sws.append(sw)
                        for hh in range(2):
                            sw = sws[hh]
                            if j == 0:
                                nc.tensor.matmul(sw[0:128, 0:s],
                                                 identb[:],
                                                 madd_g16[:, 0:s],
                                                 start=False, stop=False)
                            if j >= 4 * Q:   # causal diagonal block
                                nc.tensor.matmul(sw[0:128, 0:128],
                                                 identb[:], madd_diag[:],
                                                 start=False, stop=True)
                            else:            # window lower-edge block
                                nc.tensor.matmul(sw[0:128, s - 128:s],
                                                 identb[:], madd_edge[:],
                                                 start=False, stop=True)
                            pt = pbc.tile([128, 512], bf16, tag="pt",
                                          bufs=8)
                            nc.scalar.activation(pt[:, 0:s],
                                                 sw[0:128, 0:s],
                                                 AF.Exp, scale=SCALE)
                            pts[(j, hh)] = (pt, s, qlo)

                    def pv2(j):
                        for hh in range(2):
                            pt, s, qlo = pts.pop((j, hh))
                            o = 128 * (qlo - 4 * Q)
                            nc.tensor.matmul(yqs[hh][0:HD + 1, o:o + s],
                                             vgs[j][:], pt[:, 0:s],
                                             start=False,
                                             stop=(j == 4 * Q + 3))

                    for idx in range(min(DEPTH, len(js))):
                        scores2(js[idx])
                    for hh in range(2):
                        nc.tensor.matmul(yqs[hh][0:HD + 1, :],
                                         vgs[0][0:N_GLOBAL, :],
                                         pgs[hh][:],
                                         start=True, stop=False)
                    for idx, j in enumerate(js):
                        if idx + DEPTH < len(js):
                            scores2(js[idx + DEPTH])
                        pv2(j)
                    for hh in range(2):
                        h = 2 * p + hh
                        nc.vector.tensor_copy(
                            den[0:1, 512 * h:512 * (h + 1)],
                            yqs[hh][HD:HD + 1, :])
                        yu = pbc.tile([64, 512], f32, tag=f"yu{h}",
                                      bufs=1, name=f"yu{h}")
                        nc.vector.tensor_copy(yu[:], yqs[hh][0:HD, :])
                        yus.append(yu)

                def wo_load(Q, pc):
                    # one batched 2MB DMA on the gpsimd queue: gates on
                    # AG(Q) completion and must not block the sync
                    # queue's xt loads for the next chunk
                    ytb = pc.tile([128, 16 * 512], bf16, tag="yt",
                                  bufs=2, name="yt")
                    nc.gpsimd.dma_start(
                        out=ytb[:].rearrange("p (g c) -> p g c", g=16),
                        in_=ago[Q][:].rearrange("(g p) c -> p g c",
                                                g=16))
                    return ytb

                def wo_chunk(Q, ytb, pc):
                    c0 = 512 * Q
                    for ob in range(2):
                        wp = ps_tile([128, 512], ("wp", "ms")[ob])
                        for ci in range(16):
                            nc.tensor.matmul(
                                wp[0:128, :],
                                wo_sb[ci][:, 128 * ob:128 * (ob + 1)],
                                ytb[:, 512 * ci:512 * (ci + 1)],
                                start=(ci == 0), stop=(ci == 15))
                        ot = pc.tile([128, 512], f32, tag="ot", bufs=3)
                        nc.scalar.copy(ot[:], wp[0:128, :])
                        nc.sync.dma_start(
                            out=outT[128 * ob:128 * (ob + 1),
                                     c0:c0 + 512],
                            in_=ot[:])

                # ================= fused chunk pipeline =================
                with tc.tile_pool(name="pa", bufs=1) as pa:

                    for j in range(NB):
                        nc.gpsimd.tensor_copy(vgs[j][:, HD:HD + 1],
                                              ones1[:])

                    yt_pref = {}
                    for Q in range(NSB):
                        c0 = 512 * Q
                        vt = qkv_chunk(Q, pa)
                        if Q == 3:
                            # before AG(3)'s trigger reaches gpsimd (the
                            # trigger blocks the queue until the AG is
                            # done); AG(2) completes early in this chunk
                            yt_pref[2] = wo_load(2, pa)
                        if Q == 3:
                            # lag-3: first consumer of gathered y sits a
                            # full 3 chunks behind AG(0)'s trigger, so core
                            # launch skew rarely stalls the PE here
                            wo_chunk(0, yt_pref.pop(0), pa)
                        den = pbc.tile([1, NH_LOC * 512], f32, tag="den",
                                       bufs=1)
                        yus = []
                        attn_pair(0, Q, den, yus)
                        if Q == 3:
                            wo_chunk(1, yt_pref.pop(1), pa)
                        attn_pair(1, Q, den, yus)
                        rden = pbc.tile([1, NH_LOC * 512], f32, tag="rden",
                                        bufs=1)
                        nc.vector.reciprocal_approx_fast(rden[:], den[:])
                        ytn = pbc.tile([64, NH_LOC * 512], bf16,
                                       tag="ytn", bufs=2, name="ytn")
                        for h in range(NH_LOC):
                            rb = pbc.tile([64, 512], f32, tag="rb", bufs=2)
                            nc.gpsimd.partition_broadcast(
                                rb[:], rden[0:1, 512 * h:512 * (h + 1)])
                            nc.vector.tensor_mul(
                                ytn[:, 512 * h:512 * (h + 1)],
                                yus[h][:], rb[:])
                        # one batched agi write on the scalar queue (idle
                        # post-exp; keeps sync free for the next xt load)
                        nc.scalar.dma_start(
                            out=agi[Q][:].rearrange("(g p) c -> p g c",
                                                    g=NH_LOC),
                            in_=ytn[:].rearrange("p (g c) -> p g c",
                                                 g=NH_LOC))
                        nc.gpsimd.collective_compute(
                            "AllGather", mybir.AluOpType.bypass,
                            replica_groups=[list(range(N_CORES))],
                            ins=[agi[Q][:]], outs=[ago[Q][:]])
                        if Q == 2:
                            yt_pref[0] = wo_load(0, pa)
                            yt_pref[1] = wo_load(1, pa)
                    wo_chunk(2, yt_pref.pop(2), pa)
                    yt_pref[3] = wo_load(3, pa)
                    wo_chunk(3, yt_pref.pop(3), pa)

    nc.compile()
    return nc


_PERM = np.concatenate([np.arange(0, HD, 2), np.arange(1, HD, 2)])

# gathered-y row order is h-major: row 512h + 64c + d holds global channel
# 256c + 64h + d; permute wo's input dims to match
_CI_PERM = np.empty(C, np.int64)
for _h in range(NH_LOC):
    for _c in range(N_CORES):
        _CI_PERM[512 * _h + 64 * _c:512 * _h + 64 * _c + 64] = \
            np.arange(256 * _c + 64 * _h, 256 * _c + 64 * _h + 64)


def _prep_inputs(x, freqs_cos, freqs_sin, wq, wk, wv, wo):
    import ml_dtypes
    bf = ml_dtypes.bfloat16
    x = np.asarray(x, np.float32)
    wq = np.asarray(wq, np.float32)
    wk = np.asarray(wk, np.float32)
    wv = np.asarray(wv, np.float32)
    wo = np.asarray(wo, np.float32)
    fc = np.asarray(freqs_cos, np.float32).T   # [32, T]
    fs = np.asarray(freqs_sin, np.float32).T

    xT = np.ascontiguousarray(x[0].T).astype(bf)          # [C, T]
    cs128 = np.ascontiguousarray(np.concatenate([fc, fc, fc, fc], axis=0))
    sn128 = np.ascontiguousarray(np.concatenate([-fs, fs, -fs, fs], axis=0))

    in_maps = []
    for c in range(N_CORES):
        wq_c = wq[QD * c:QD * (c + 1), :].reshape(NH_LOC, HD, C)
        wq_c = wq_c[:, _PERM, :].reshape(QD, C)
        wk_c = wk[HD * c:HD * (c + 1), :][_PERM, :]
        wv_c = wv[HD * c:HD * (c + 1), :]
        in_maps.append({
            "xT": xT,
            "wqT": np.ascontiguousarray(wq_c.T).astype(bf),
            "wkvT": np.ascontiguousarray(
                np.concatenate([wk_c.T, wv_c.T], axis=1)).astype(bf),
            "woT": np.ascontiguousarray(
                wo[QD * c:QD * (c + 1), :].T).astype(bf),
            "cs128": cs128,
            "sn128": sn128,
        })
    return in_maps


def get_nc():
    if "nc" not in _CACHE:
        _CACHE["nc"] = _build()
    return _CACHE["nc"]


def kernel(x, freqs_cos, freqs_sin, wq, wk, wv, wo, **run_kwargs):
    from concourse.bass_utils import run_bass_kernel_spmd
    nc = get_nc()
    in_maps = _prep_inputs(x, freqs_cos, freqs_sin, wq, wk, wv, wo)
    res = run_bass_kernel_spmd(nc, in_maps, list(range(N_CORES)), **run_kwargs)
    outT = np.concatenate([res.results[c]["outT"] for c in range(N_CORES)],
                          axis=0)
    out = np.ascontiguousarray(outT.T).reshape(1, T, C).astype(np.float32)
    if run_kwargs:
        kernel.last_results = res
    return out


# revision 45
# speedup vs baseline: 1.1021x; 1.1021x over previous
"""GQA sliding-window attention (B=1, T=2048, C=2048, 32 Q / 8 KV heads,
head_dim=64, window=512, 16 global tokens) on 8 Trainium2 NeuronCores.

Sharding: tensor-parallel over heads — core c owns KV head c and Q heads
4c..4c+3.  Everything on-device runs transposed ([feature, token] layout).

v2 layout vs baseline:
  * fully fused per-512-token-chunk pipeline: QKV+RoPE(i) -> WO(i-2) ->
    attention(i) -> AllGather(i) trigger -> prefetch WO inputs (i-1).
    Keeps the PE dense (HAM stays warm) and hides each chunk's AllGather
    behind the next chunk's compute.
  * bf16 for everything DMA'd or collective'd: x, wq/wkv/wo, probs, V,
    y, the AllGather payload and the WO matmul inputs.  q/k stay f32r
    in SBUF (free: same PE rate at moving>=256, no DMA cost).
  * softmax denominator reciprocal via one custom-DVE op
    (reciprocal_approx_fast) instead of scalar Ln+Exp — no ACT table
    thrash.
"""

import sys
sys.path.insert(0, "/opt/trn_rl_repo")

import numpy as np

import concourse.bass as bass
import concourse.mybir as mybir
from concourse import bacc
from concourse.tile import TileContext
from concourse.masks import make_identity

f32 = mybir.dt.float32
f32r = mybir.dt.float32r
bf16 = mybir.dt.bfloat16
AF = mybir.ActivationFunctionType

N_CORES = 8
T = 2048
C = 2048
HD = 64
NH_LOC = 4            # query heads per core
QD = NH_LOC * HD      # 256 per-core query dims
NB = T // 128         # 16 token blocks
NSB = T // 512        # 4 superblocks
N_GLOBAL = 16
SCALE = 0.125         # 1/sqrt(64)

_CACHE = {}


def _build():
    nc = bacc.Bacc(num_devices=N_CORES)

    xT = nc.declare_dram_parameter("xT", [C, T], bf16, isOutput=False)
    wqT = nc.declare_dram_parameter("wqT", [C, QD], bf16, isOutput=False)
    wkvT = nc.declare_dram_parameter("wkvT", [C, 128], bf16, isOutput=False)
    woT = nc.declare_dram_parameter("woT", [C, QD], bf16, isOutput=False)
    cs1 = nc.declare_dram_parameter("cs128", [128, T], f32, isOutput=False)
    sn1 = nc.declare_dram_parameter("sn128", [128, T], f32, isOutput=False)
    outT = nc.declare_dram_parameter("outT", [QD, T], f32, isOutput=True)

    with TileContext(nc) as tc:
        with tc.tile_pool(name="persist", bufs=1) as pp, \
             tc.tile_pool(name="psAll", bufs=1, space="PSUM") as psA, \
             tc.tile_pool(name="pdram", bufs=1, space="DRAM") as pdr:
            # ---- persistent state ----
            qTr01 = pp.tile([128, T], bf16)   # rotated q, heads 0,1
            qTr23 = pp.tile([128, T], bf16)   # rotated q, heads 2,3
            kTr2 = pp.tile([128, T], bf16)    # rotated k duplicated on halves
            vgs = [pp.tile([128, HD + 1], bf16, name=f"vg{j}", tag=f"vg{j}")
                   for j in range(NB)]
            csb = pp.tile([128, T], f32)
            snb = pp.tile([128, T], f32)
            identb = pp.tile([128, 128], bf16)
            m_diag = pp.tile([128, 128], f32)
            m_edge = pp.tile([128, 128], f32)
            mg16 = pp.tile([128, 512], f32)
            madd_diag = pp.tile([128, 128], bf16)
            madd_edge = pp.tile([128, 128], bf16)
            madd_g16 = pp.tile([128, 512], bf16)
            ones1 = pp.tile([128, 1], f32)
            wq_sb = [pp.tile([128, QD], bf16, name=f"wq{k}", tag=f"wq{k}")
                     for k in range(16)]
            wkv_sb = [pp.tile([128, 128], bf16, name=f"wkv{k}", tag=f"wkv{k}")
                      for k in range(16)]
            wo_sb = [pp.tile([128, QD], bf16, name=f"wo{k}", tag=f"wo{k}")
                     for k in range(16)]

            # additive score masks, folded into the scores psum via one
            # extra PE matmul (identb.T @ madd_*): 0 keeps, -1e5 kills
            # (exp underflows to exactly 0).  scoresT layout:
            # partition=key, free=query; diag keeps p <= f (causal),
            # edge keeps p > f (window lower edge), g16 kills window
            # copies of global keys (partitions < 16) in k-block 0.
            NEG = -1.0e5
            ib_f = pp.tile([128, 128], f32)
            make_identity(nc, ib_f[:])
            nc.gpsimd.tensor_copy(identb[:], ib_f[:])
            nc.gpsimd.memset(m_diag[:], 0.0)
            nc.gpsimd.affine_select(out=m_diag[:], in_=m_diag[:],
                                    compare_op=mybir.AluOpType.is_ge,
                                    fill=NEG,
                                    base=0, pattern=[[1, 128]],
                                    channel_multiplier=-1)
            nc.gpsimd.memset(m_edge[:], 0.0)
            nc.gpsimd.affine_select(out=m_edge[:], in_=m_edge[:],
                                    compare_op=mybir.AluOpType.is_ge,
                                    fill=NEG,
                                    base=-1, pattern=[[-1, 128]],
                                    channel_multiplier=1)
            nc.gpsimd.memset(mg16[:], 0.0)
            nc.gpsimd.affine_select(out=mg16[:], in_=mg16[:],
                                    compare_op=mybir.AluOpType.is_ge,
                                    fill=NEG,
                                    base=-N_GLOBAL, pattern=[[0, 512]],
                                    channel_multiplier=1)
            nc.vector.memset(ones1[:], 1.0)
            nc.gpsimd.tensor_copy(madd_diag[:], m_diag[:])
            nc.gpsimd.tensor_copy(madd_edge[:], m_edge[:])
            nc.gpsimd.tensor_copy(madd_g16[:], mg16[:])

            # psum banks (8):
            #   q01/q23/kv: QKV accum, then score-block rotation slots
            #   swd: 4th score rotation slot
            #   yq (bufs=2): PV accum ping-pong across heads
            #   ms: vT transposes + global scores + 2nd WO bank
            #   wp: WO accum
            def ps_tile(shape, tag, bufs=1):
                return psA.tile(shape, f32, tag=tag, bufs=bufs, name=tag,
                                padded_shape=[128, 512])

            agi = [pdr.tile([QD, 512], bf16, name=f"agi{Q}", tag=f"agi{Q}")
                   for Q in range(NSB)]
            ago = [pdr.tile([C, 512], bf16, name=f"ago{Q}", tag=f"ago{Q}",
                            addr_space="Shared") for Q in range(NSB)]

            with tc.tile_pool(name="pbc", bufs=1) as pbc:

                def rope(dst, psrc, rows, col0, pa):
                    """dst[0:rows, col0:+512] = rope(psrc); head dims are
                    pre-permuted to rotate-half order (evens then odds).
                    The partner-swap reads psum at a 32-partition offset
                    directly — no staging copy."""
                    t1 = pa.tile([rows, 512], f32, tag="t1", bufs=2)
                    t2 = pa.tile([rows, 512], f32, tag="t2", bufs=2)
                    nc.vector.tensor_mul(t1[:], psrc,
                                         csb[0:rows, col0:col0 + 512])
                    for b in range(rows // 32):
                        s = b ^ 1
                        nc.vector.tensor_mul(
                            t2[32 * b:32 * b + 32, :],
                            psrc[32 * s:32 * s + 32, :],
                            snb[32 * b:32 * b + 32, col0:col0 + 512])
                    nc.vector.tensor_add(dst[0:rows, col0:col0 + 512],
                                         t1[:], t2[:])

                def qkv_chunk(tcc, pa):
                    c0 = 512 * tcc
                    q01 = ps_tile([128, 512], "q01")
                    q23 = ps_tile([128, 512], "q23")
                    kv = ps_tile([128, 512], "kv")
                    # one batched 2MB DMA for the whole chunk of x
                    # (k-tile kt lands at columns 512*kt)
                    xtb = pa.tile([128, 16 * 512], bf16, tag="xt", bufs=2)
                    nc.sync.dma_start(
                        out=xtb[:].rearrange("p (g c) -> p g c", g=16),
                        in_=xT[:, c0:c0 + 512].rearrange(
                            "(g p) c -> p g c", g=16))
                    if tcc == 0:
                        for kt in range(16):
                            nc.sync.dma_start(
                                out=wq_sb[kt][:],
                                in_=wqT[128 * kt:128 * (kt + 1), :])
                            nc.sync.dma_start(
                                out=wkv_sb[kt][:],
                                in_=wkvT[128 * kt:128 * (kt + 1), :])
                        nc.sync.dma_start(out=csb[:], in_=cs1[:])
                        nc.sync.dma_start(out=snb[:], in_=sn1[:])
                    if tcc == 0:
                        # wo weights on the gpsimd queue, emitted before
                        # AG(0)'s trigger so they never queue behind a
                        # collective wait; sync stays free for xt
                        for kt in range(16):
                            nc.gpsimd.dma_start(
                                out=wo_sb[kt][:],
                                in_=woT[128 * kt:128 * (kt + 1), :])
                    for kt in range(16):
                        xt = xtb[:, 512 * kt:512 * (kt + 1)]
                        st, sp = kt == 0, kt == 15
                        nc.tensor.matmul(q01[0:128, :],
                                         wq_sb[kt][:, 0:128],
                                         xt, start=st, stop=sp)
                        nc.tensor.matmul(q23[0:128, :],
                                         wq_sb[kt][:, 128:256],
                                         xt, start=st, stop=sp)
                        nc.tensor.matmul(kv[0:128, :], wkv_sb[kt][:],
                                         xt, start=st, stop=sp)
                    rope(qTr01, q01[0:128, :], 128, c0, pa)
                    rope(qTr23, q23[0:128, :], 128, c0, pa)
                    rope(kTr2, kv[0:64, :], 64, c0, pa)
                    for b in range(2):
                        nc.vector.tensor_copy(
                            kTr2[64 + 32 * b:96 + 32 * b, c0:c0 + 512],
                            kTr2[32 * b:32 * (b + 1), c0:c0 + 512])
                    vt = pa.tile([64, 512], bf16, tag="vt", bufs=2)
                    # scalar copy: frees the vgs transpose DMAs from the
                    # DVE rope queue (scalar is idle until the first exp)
                    nc.scalar.copy(vt[:], kv[64:128, :])
                    # vgs via the DMA xbar transpose engine: keeps the PE
                    # and DVE queues out of the v-transpose dependency
                    # (ones columns pre-written once at startup)
                    for jj in range(4):
                        j = 4 * tcc + jj
                        nc.sync.dma_start_transpose(
                            out=vgs[j][:, 0:HD],
                            in_=vt[:, 128 * jj:128 * (jj + 1)])

                def attn_pair(p, Q, den, yus):
                    """heads (2p, 2p+1): head 2p on array rows 0:64, head
                    2p+1 on rows 64:128 (kTr2 duplicated on both halves
                    for this) — paired score matmuls target different
                    row groups and execute concurrently on the PE."""
                    qt = qTr01 if p == 0 else qTr23
                    c0 = 512 * Q
                    yqs = [ps_tile([HD + 1, 512], "yq", bufs=2)
                           for _ in range(2)]
                    pgs = []
                    for hh in range(2):
                        qb = 64 * hh
                        sg = ps_tile([N_GLOBAL, 512], "ms")
                        nc.tensor.matmul(sg[0:N_GLOBAL, :],
                                         kTr2[qb:qb + 64, 0:N_GLOBAL],
                                         qt[qb:qb + 64, c0:c0 + 512],
                                         start=True, stop=True)
                        pg = pbc.tile([N_GLOBAL, 512], bf16,
                                      tag=f"pg{hh}", bufs=1)
                        nc.scalar.activation(pg[:], sg[0:N_GLOBAL, :],
                                             AF.Exp, scale=SCALE)
                        pgs.append(pg)
                    js = list(range(max(0, 4 * Q - 4), 4 * Q + 4))
                    DEPTH = 2
                    pts = {}

                    def scores2(j):
                        qlo = max(4 * Q, j)
                        qhi = min(4 * Q + 3, j + 4)
                        s = 128 * (qhi - qlo + 1)
                        sws = []
                        for hh in range(2):
                            qb = 64 * hh
                            sw = ps_tile([128, s],
                                         ("q01", "q23", "kv", "swd")[
                                             (2 * j + hh) % 4])
                            nc.tensor.matmul(
                                sw[0:128, 0:s],
                                kTr2[qb:qb + 64, 128 * j:128 * (j + 1)],
                                qt[qb:qb + 64, 128 * qlo:128 * qlo + s],
                                start=True, stop=False)
                            sws.append(sw)
                        for hh in range(2):
                            sw = sws[hh]
                            if j == 0:
                                nc.tensor.matmul(sw[0:128, 0:s],
                                                 identb[:],
                                                 madd_g16[:, 0:s],
                                                 start=False, stop=False)
                            if j >= 4 * Q:   # causal diagonal block
                                nc.tensor.matmul(sw[0:128, 0:128],
                                                 identb[:], madd_diag[:],
                                                 start=False, stop=True)
                            else:            # window lower-edge block
                                nc.tensor.matmul(sw[0:128, s - 128:s],
                                                 identb[:], madd_edge[:],
                                                 start=False, stop=True)
                            pt = pbc.tile([128, 512], bf16, tag="pt",
                                          bufs=8)
                            nc.scalar.activation(pt[:, 0:s],
                                                 sw[0:128, 0:s],
                                                 AF.Exp, scale=SCALE)
                            pts[(j, hh)] = (pt, s, qlo)

                    def pv2(j):
                        for hh in range(2):
                            pt, s, qlo = pts.pop((j, hh))
                            o = 128 * (qlo - 4 * Q)
                            nc.tensor.matmul(yqs[hh][0:HD + 1, o:o + s],
                                             vgs[j][:], pt[:, 0:s],
                                             start=False,
                                             stop=(j == 4 * Q + 3))

                    for idx in range(min(DEPTH, len(js))):
                        scores2(js[idx])
                    for hh in range(2):
                        nc.tensor.matmul(yqs[hh][0:HD + 1, :],
                                         vgs[0][0:N_GLOBAL, :],
                                         pgs[hh][:],
                                         start=True, stop=False)
                    for idx, j in enumerate(js):
                        if idx + DEPTH < len(js):
                            scores2(js[idx + DEPTH])
                        pv2(j)
                    for hh in range(2):
                        h = 2 * p + hh
                        nc.vector.tensor_copy(
                            den[0:1, 512 * h:512 * (h + 1)],
                            yqs[hh][HD:HD + 1, :])
                        yu = pbc.tile([64, 512], f32, tag=f"yu{h}",
                                      bufs=1, name=f"yu{h}")
                        nc.vector.tensor_copy(yu[:], yqs[hh][0:HD, :])
                        yus.append(yu)

                def wo_load(Q, pc, eng=None):
                    # one batched 2MB DMA, default on the gpsimd queue:
                    # gates on AG(Q) completion and must not block the
                    # sync queue's xt loads for the next chunk.  The tail
                    # loads go on sync instead — gpsimd is blocked there
                    # by the AG(3) trigger until the collective finishes,
                    # while sync has nothing left to do.
                    eng = eng or nc.gpsimd
                    ytb = pc.tile([128, 16 * 512], bf16, tag="yt",
                                  bufs=2, name="yt")
                    eng.dma_start(
                        out=ytb[:].rearrange("p (g c) -> p g c", g=16),
                        in_=ago[Q][:].rearrange("(g p) c -> p g c",
                                                g=16))
                    return ytb

                def wo_chunk(Q, ytb, pc):
                    c0 = 512 * Q
                    for ob in range(2):
                        wp = ps_tile([128, 512], ("wp", "ms")[ob])
                        for ci in range(16):
                            nc.tensor.matmul(
                                wp[0:128, :],
                                wo_sb[ci][:, 128 * ob:128 * (ob + 1)],
                                ytb[:, 512 * ci:512 * (ci + 1)],
                                start=(ci == 0), stop=(ci == 15))
                        ot = pc.tile([128, 512], f32, tag="ot", bufs=3)
                        nc.scalar.copy(ot[:], wp[0:128, :])
                        nc.sync.dma_start(
                            out=outT[128 * ob:128 * (ob + 1),
                                     c0:c0 + 512],
                            in_=ot[:])

                # ================= fused chunk pipeline =================
                with tc.tile_pool(name="pa", bufs=1) as pa:

                    for j in range(NB):
                        nc.gpsimd.tensor_copy(vgs[j][:, HD:HD + 1],
                                              ones1[:])

                    yt_pref = {}
                    for Q in range(NSB):
                        c0 = 512 * Q
                        vt = qkv_chunk(Q, pa)
                        if Q == 3:
                            # lag-3: first consumer of gathered y sits a
                            # full 3 chunks behind AG(0)'s trigger, so core
                            # launch skew rarely stalls the PE here
                            wo_chunk(0, yt_pref.pop(0), pa)
                        den = pbc.tile([1, NH_LOC * 512], f32, tag="den",
                                       bufs=1)
                        yus = []
                        attn_pair(0, Q, den, yus)
                        if Q == 3:
                            wo_chunk(1, yt_pref.pop(1), pa)
                        attn_pair(1, Q, den, yus)
                        rden = pbc.tile([1, NH_LOC * 512], f32, tag="rden",
                                        bufs=1)
                        nc.vector.reciprocal_approx_fast(rden[:], den[:])
                        ytn = pbc.tile([64, NH_LOC * 512], bf16,
                                       tag="ytn", bufs=2, name="ytn")
                        for h in range(NH_LOC):
                            rb = pbc.tile([64, 512], f32, tag="rb", bufs=2)
                            nc.gpsimd.partition_broadcast(
                                rb[:], rden[0:1, 512 * h:512 * (h + 1)])
                            nc.vector.tensor_mul(
                                ytn[:, 512 * h:512 * (h + 1)],
                                yus[h][:], rb[:])
                        # one batched agi write on the scalar queue (idle
                        # post-exp; keeps sync free for the next xt load)
                        nc.scalar.dma_start(
                            out=agi[Q][:].rearrange("(g p) c -> p g c",
                                                    g=NH_LOC),
                            in_=ytn[:].rearrange("p (g c) -> p g c",
                                                 g=NH_LOC))
                        nc.gpsimd.collective_compute(
                            "AllGather", mybir.AluOpType.bypass,
                            replica_groups=[list(range(N_CORES))],
                            ins=[agi[Q][:]], outs=[ago[Q][:]])
                        if Q == 2:
                            yt_pref[0] = wo_load(0, pa)
                            yt_pref[1] = wo_load(1, pa)
                        if Q == 3:
                            yt_pref[2] = wo_load(2, pa, eng=nc.sync)
                    wo_chunk(2, yt_pref.pop(2), pa)
                    yt_pref[3] = wo_load(3, pa, eng=nc.sync)
                    wo_chunk(3, yt_pref.pop(3), pa)

    nc.compile()
    return nc


_PERM = np.concatenate([np.arange(0, HD, 2), np.arange(1, HD, 2)])

# gathered-y row order is h-major: row 512h + 64c + d holds global channel
# 256c + 64h + d; permute wo's input dims to match
_CI_PERM = np.empty(C, np.int64)
for _h in range(NH_LOC):
    for _c in range(N_CORES):
        _CI_PERM[512 * _h + 64 * _c:512 * _h + 64 * _c + 64] = \
            np.arange(256 * _c + 64 * _h, 256 * _c + 64 * _h + 64)


def _prep_inputs(x, freqs_cos, freqs_sin, wq, wk, wv, wo):
    import ml_dtypes
    bf = ml_dtypes.bfloat16
    x = np.asarray(x, np.float32)
    wq = np.asarray(wq, np.float32)
    wk = np.asarray(wk, np.float32)
    wv = np.asarray(wv, np.float32)
    wo = np.asarray(wo, np.float32)
    fc = np.asarray(freqs_cos, np.float32).T   # [32, T]
    fs = np.asarray(freqs_sin, np.float32).T

    xT = np.ascontiguousarray(x[0].T).astype(bf)          # [C, T]
    cs128 = np.ascontiguousarray(np.concatenate([fc, fc, fc, fc], axis=0))
    sn128 = np.ascontiguousarray(np.concatenate([-fs, fs, -fs, fs], axis=0))

    in_maps = []
    for c in range(N_CORES):
        wq_c = wq[QD * c:QD * (c + 1), :].reshape(NH_LOC, HD, C)
        wq_c = wq_c[:, _PERM, :].reshape(QD, C)
        wk_c = wk[HD * c:HD * (c + 1), :][_PERM, :]
        wv_c = wv[HD * c:HD * (c + 1), :]
        in_maps.append({
            "xT": xT,
            "wqT": np.ascontiguousarray(wq_c.T).astype(bf),
            "wkvT": np.ascontiguousarray(
                np.concatenate([wk_c.T, wv_c.T], axis=1)).astype(bf),
            "woT": np.ascontiguousarray(
                wo[QD * c:QD * (c + 1), :].T).astype(bf),
            "cs128": cs128,
            "sn128": sn128,
        })
    return in_maps


def get_nc():
    if "nc" not in _CACHE:
        _CACHE["nc"] = _build()
    return _CACHE["nc"]


def kernel(x, freqs_cos, freqs_sin, wq, wk, wv, wo, **run_kwargs):
    from concourse.bass_utils import run_bass_kernel_spmd
    nc = get_nc()
    in_maps = _prep_inputs(x, freqs_cos, freqs_sin, wq, wk, wv, wo)
    res = run_bass_kernel_spmd(nc, in_maps, list(range(N_CORES)), **run_kwargs)
    outT = np.concatenate([res.results[c]["outT"] for c in range(N_CORES)],
                          axis=0)
    out = np.ascontiguousarray(outT.T).reshape(1, T, C).astype(np.float32)
    if run_kwargs:
        kernel.last_results = res
    return out


# revision 46
# speedup vs baseline: 1.1866x; 1.0767x over previous
"""GQA sliding-window attention (B=1, T=2048, C=2048, 32 Q / 8 KV heads,
head_dim=64, window=512, 16 global tokens) on 8 Trainium2 NeuronCores.

Sharding: tensor-parallel over heads — core c owns KV head c and Q heads
4c..4c+3.  Everything on-device runs transposed ([feature, token] layout).

v2 layout vs baseline:
  * fully fused per-512-token-chunk pipeline: QKV+RoPE(i) -> WO(i-2) ->
    attention(i) -> AllGather(i) trigger -> prefetch WO inputs (i-1).
    Keeps the PE dense (HAM stays warm) and hides each chunk's AllGather
    behind the next chunk's compute.
  * bf16 for everything DMA'd or collective'd: x, wq/wkv/wo, probs, V,
    y, the AllGather payload and the WO matmul inputs.  q/k stay f32r
    in SBUF (free: same PE rate at moving>=256, no DMA cost).
  * softmax denominator reciprocal via one custom-DVE op
    (reciprocal_approx_fast) instead of scalar Ln+Exp — no ACT table
    thrash.
"""

import sys
sys.path.insert(0, "/opt/trn_rl_repo")

import numpy as np

import concourse.bass as bass
import concourse.mybir as mybir
from concourse import bacc
from concourse.tile import TileContext
from concourse.masks import make_identity

f32 = mybir.dt.float32
f32r = mybir.dt.float32r
bf16 = mybir.dt.bfloat16
AF = mybir.ActivationFunctionType

N_CORES = 8
T = 2048
C = 2048
HD = 64
NH_LOC = 4            # query heads per core
QD = NH_LOC * HD      # 256 per-core query dims
NB = T // 128         # 16 token blocks
NSB = T // 512        # 4 superblocks
N_GLOBAL = 16
SCALE = 0.125         # 1/sqrt(64)

_CACHE = {}


def _build():
    nc = bacc.Bacc(num_devices=N_CORES)

    xT = nc.declare_dram_parameter("xT", [C, T], bf16, isOutput=False)
    wqT = nc.declare_dram_parameter("wqT", [C, QD], bf16, isOutput=False)
    wkvT = nc.declare_dram_parameter("wkvT", [C, 128], bf16, isOutput=False)
    woT = nc.declare_dram_parameter("woT", [C, QD], bf16, isOutput=False)
    cs1 = nc.declare_dram_parameter("cs128", [128, T], f32, isOutput=False)
    sn1 = nc.declare_dram_parameter("sn128", [128, T], f32, isOutput=False)
    outT = nc.declare_dram_parameter("outT", [QD, T], f32, isOutput=True)

    with TileContext(nc) as tc:
        with tc.tile_pool(name="persist", bufs=1) as pp, \
             tc.tile_pool(name="psAll", bufs=1, space="PSUM") as psA, \
             tc.tile_pool(name="pdram", bufs=1, space="DRAM") as pdr:
            # ---- persistent state ----
            qTr01 = pp.tile([128, T], bf16)   # rotated q, heads 0,1
            qTr23 = pp.tile([128, T], bf16)   # rotated q, heads 2,3
            kTr2 = pp.tile([128, T], bf16)    # rotated k duplicated on halves
            vgs = [pp.tile([128, HD + 1], bf16, name=f"vg{j}", tag=f"vg{j}")
                   for j in range(NB)]
            csb = pp.tile([128, T], f32)
            snb = pp.tile([128, T], f32)
            identb = pp.tile([128, 128], bf16)
            m_diag = pp.tile([128, 128], f32)
            m_edge = pp.tile([128, 128], f32)
            mg16 = pp.tile([128, 512], f32)
            madd_diag = pp.tile([128, 128], bf16)
            madd_edge = pp.tile([128, 128], bf16)
            madd_g16 = pp.tile([128, 512], bf16)
            ones1 = pp.tile([128, 1], f32)
            wq_sb = [pp.tile([128, QD], bf16, name=f"wq{k}", tag=f"wq{k}")
                     for k in range(16)]
            wkv_sb = [pp.tile([128, 128], bf16, name=f"wkv{k}", tag=f"wkv{k}")
                      for k in range(16)]
            wo_sb = [pp.tile([128, QD], bf16, name=f"wo{k}", tag=f"wo{k}")
                     for k in range(16)]

            # additive score masks, folded into the scores psum via one
            # extra PE matmul (identb.T @ madd_*): 0 keeps, -1e5 kills
            # (exp underflows to exactly 0).  scoresT layout:
            # partition=key, free=query; diag keeps p <= f (causal),
            # edge keeps p > f (window lower edge), g16 kills window
            # copies of global keys (partitions < 16) in k-block 0.
            NEG = -1.0e5
            ib_f = pp.tile([128, 128], f32)
            make_identity(nc, ib_f[:])
            nc.gpsimd.tensor_copy(identb[:], ib_f[:])
            nc.gpsimd.memset(m_diag[:], 0.0)
            nc.gpsimd.affine_select(out=m_diag[:], in_=m_diag[:],
                                    compare_op=mybir.AluOpType.is_ge,
                                    fill=NEG,
                                    base=0, pattern=[[1, 128]],
                                    channel_multiplier=-1)
            nc.gpsimd.memset(m_edge[:], 0.0)
            nc.gpsimd.affine_select(out=m_edge[:], in_=m_edge[:],
                                    compare_op=mybir.AluOpType.is_ge,
                                    fill=NEG,
                                    base=-1, pattern=[[-1, 128]],
                                    channel_multiplier=1)
            nc.gpsimd.memset(mg16[:], 0.0)
            nc.gpsimd.affine_select(out=mg16[:], in_=mg16[:],
                                    compare_op=mybir.AluOpType.is_ge,
                                    fill=NEG,
                                    base=-N_GLOBAL, pattern=[[0, 512]],
                                    channel_multiplier=1)
            nc.vector.memset(ones1[:], 1.0)
            nc.gpsimd.tensor_copy(madd_diag[:], m_diag[:])
            nc.gpsimd.tensor_copy(madd_edge[:], m_edge[:])
            nc.gpsimd.tensor_copy(madd_g16[:], mg16[:])

            # psum banks (8):
            #   q01/q23/kv: QKV accum, then score-block rotation slots
            #   swd: 4th score rotation slot
            #   yq (bufs=2): PV accum ping-pong across heads
            #   ms: vT transposes + global scores + 2nd WO bank
            #   wp: WO accum
            def ps_tile(shape, tag, bufs=1):
                return psA.tile(shape, f32, tag=tag, bufs=bufs, name=tag,
                                padded_shape=[128, 512])

            agi = [pdr.tile([QD, 512], bf16, name=f"agi{Q}", tag=f"agi{Q}")
                   for Q in range(NSB)]
            ago = [pdr.tile([C, 512], bf16, name=f"ago{Q}", tag=f"ago{Q}",
                            addr_space="Shared") for Q in range(NSB)]

            with tc.tile_pool(name="pbc", bufs=1) as pbc:

                def rope(dst, psrc, rows, col0, pa):
                    """dst[0:rows, col0:+512] = rope(psrc); head dims are
                    pre-permuted to rotate-half order (evens then odds).
                    The partner-swap reads psum at a 32-partition offset
                    directly — no staging copy."""
                    t1 = pa.tile([rows, 512], f32, tag="t1", bufs=2)
                    t2 = pa.tile([rows, 512], f32, tag="t2", bufs=2)
                    nc.vector.tensor_mul(t1[:], psrc,
                                         csb[0:rows, col0:col0 + 512])
                    for b in range(rows // 32):
                        s = b ^ 1
                        nc.vector.tensor_mul(
                            t2[32 * b:32 * b + 32, :],
                            psrc[32 * s:32 * s + 32, :],
                            snb[32 * b:32 * b + 32, col0:col0 + 512])
                    nc.vector.tensor_add(dst[0:rows, col0:col0 + 512],
                                         t1[:], t2[:])

                def qkv_chunk(tcc, pa):
                    c0 = 512 * tcc
                    q01 = ps_tile([128, 512], "q01")
                    q23 = ps_tile([128, 512], "q23")
                    kv = ps_tile([128, 512], "kv")
                    # one batched 2MB DMA for the whole chunk of x
                    # (k-tile kt lands at columns 512*kt)
                    xtb = pa.tile([128, 16 * 512], bf16, tag="xt", bufs=2)
                    nc.sync.dma_start(
                        out=xtb[:].rearrange("p (g c) -> p g c", g=16),
                        in_=xT[:, c0:c0 + 512].rearrange(
                            "(g p) c -> p g c", g=16))
                    if tcc == 0:
                        for kt in range(16):
                            nc.sync.dma_start(
                                out=wq_sb[kt][:],
                                in_=wqT[128 * kt:128 * (kt + 1), :])
                            nc.sync.dma_start(
                                out=wkv_sb[kt][:],
                                in_=wkvT[128 * kt:128 * (kt + 1), :])
                        nc.sync.dma_start(out=csb[:], in_=cs1[:])
                        nc.sync.dma_start(out=snb[:], in_=sn1[:])
                    if tcc == 0:
                        # wo weights on the gpsimd queue, emitted before
                        # AG(0)'s trigger so they never queue behind a
                        # collective wait; sync stays free for xt
                        for kt in range(16):
                            nc.gpsimd.dma_start(
                                out=wo_sb[kt][:],
                                in_=woT[128 * kt:128 * (kt + 1), :])
                    for kt in range(16):
                        xt = xtb[:, 512 * kt:512 * (kt + 1)]
                        st, sp = kt == 0, kt == 15
                        nc.tensor.matmul(q01[0:128, :],
                                         wq_sb[kt][:, 0:128],
                                         xt, start=st, stop=sp)
                        nc.tensor.matmul(q23[0:128, :],
                                         wq_sb[kt][:, 128:256],
                                         xt, start=st, stop=sp)
                        nc.tensor.matmul(kv[0:128, :], wkv_sb[kt][:],
                                         xt, start=st, stop=sp)
                    rope(qTr01, q01[0:128, :], 128, c0, pa)
                    rope(qTr23, q23[0:128, :], 128, c0, pa)
                    rope(kTr2, kv[0:64, :], 64, c0, pa)
                    for b in range(2):
                        nc.vector.tensor_copy(
                            kTr2[64 + 32 * b:96 + 32 * b, c0:c0 + 512],
                            kTr2[32 * b:32 * (b + 1), c0:c0 + 512])
                    vt = pa.tile([64, 512], bf16, tag="vt", bufs=2)
                    # scalar copy: frees the vgs transpose DMAs from the
                    # DVE rope queue (scalar is idle until the first exp)
                    nc.scalar.copy(vt[:], kv[64:128, :])
                    # vgs via the DMA xbar transpose engine: keeps the PE
                    # and DVE queues out of the v-transpose dependency
                    # (ones columns pre-written once at startup)
                    for jj in range(4):
                        j = 4 * tcc + jj
                        nc.sync.dma_start_transpose(
                            out=vgs[j][:, 0:HD],
                            in_=vt[:, 128 * jj:128 * (jj + 1)])

                def attn_pair(p, Q, rdens, yus):
                    """heads (2p, 2p+1): head 2p on array rows 0:64, head
                    2p+1 on rows 64:128 (kTr2 duplicated on both halves
                    for this) — paired score matmuls target different
                    row groups and execute concurrently on the PE."""
                    qt = qTr01 if p == 0 else qTr23
                    c0 = 512 * Q
                    yqs = [ps_tile([HD + 1, 512], "yq", bufs=2)
                           for _ in range(2)]
                    pgs = []
                    for hh in range(2):
                        qb = 64 * hh
                        sg = ps_tile([N_GLOBAL, 512], "ms")
                        nc.tensor.matmul(sg[0:N_GLOBAL, :],
                                         kTr2[qb:qb + 64, 0:N_GLOBAL],
                                         qt[qb:qb + 64, c0:c0 + 512],
                                         start=True, stop=True)
                        pg = pbc.tile([N_GLOBAL, 512], bf16,
                                      tag=f"pg{hh}", bufs=1)
                        nc.scalar.activation(pg[:], sg[0:N_GLOBAL, :],
                                             AF.Exp, scale=SCALE)
                        pgs.append(pg)
                    js = list(range(max(0, 4 * Q - 4), 4 * Q + 4))
                    DEPTH = 2
                    pts = {}

                    def scores2(j):
                        qlo = max(4 * Q, j)
                        qhi = min(4 * Q + 3, j + 4)
                        s = 128 * (qhi - qlo + 1)
                        sws = []
                        for hh in range(2):
                            qb = 64 * hh
                            sw = ps_tile([128, s],
                                         ("q01", "q23", "kv", "swd")[
                                             (2 * j + hh) % 4])
                            nc.tensor.matmul(
                                sw[0:128, 0:s],
                                kTr2[qb:qb + 64, 128 * j:128 * (j + 1)],
                                qt[qb:qb + 64, 128 * qlo:128 * qlo + s],
                                start=True, stop=False)
                            sws.append(sw)
                        for hh in range(2):
                            sw = sws[hh]
                            if j == 0:
                                nc.tensor.matmul(sw[0:128, 0:s],
                                                 identb[:],
                                                 madd_g16[:, 0:s],
                                                 start=False, stop=False)
                            if j >= 4 * Q:   # causal diagonal block
                                nc.tensor.matmul(sw[0:128, 0:128],
                                                 identb[:], madd_diag[:],
                                                 start=False, stop=True)
                            else:            # window lower-edge block
                                nc.tensor.matmul(sw[0:128, s - 128:s],
                                                 identb[:], madd_edge[:],
                                                 start=False, stop=True)
                            pt = pbc.tile([128, 512], bf16, tag="pt",
                                          bufs=8)
                            nc.scalar.activation(pt[:, 0:s],
                                                 sw[0:128, 0:s],
                                                 AF.Exp, scale=SCALE)
                            pts[(j, hh)] = (pt, s, qlo)

                    def pv2(j):
                        for hh in range(2):
                            pt, s, qlo = pts.pop((j, hh))
                            o = 128 * (qlo - 4 * Q)
                            nc.tensor.matmul(yqs[hh][0:HD + 1, o:o + s],
                                             vgs[j][:], pt[:, 0:s],
                                             start=False,
                                             stop=(j == 4 * Q + 3))

                    for idx in range(min(DEPTH, len(js))):
                        scores2(js[idx])
                    for hh in range(2):
                        nc.tensor.matmul(yqs[hh][0:HD + 1, :],
                                         vgs[0][0:N_GLOBAL, :],
                                         pgs[hh][:],
                                         start=True, stop=False)
                    for idx, j in enumerate(js):
                        if idx + DEPTH < len(js):
                            scores2(js[idx + DEPTH])
                        pv2(j)
                    for hh in range(2):
                        h = 2 * p + hh
                        rd = pbc.tile([1, 512], f32, tag=f"rden{h}",
                                      bufs=1, name=f"rden{h}")
                        nc.vector.tensor_copy(rd[:],
                                              yqs[hh][HD:HD + 1, :])
                        nc.vector.reciprocal_approx_fast(rd[:], rd[:])
                        rdens.append(rd)
                        yu = pbc.tile([64, 512], f32, tag=f"yu{h}",
                                      bufs=1, name=f"yu{h}")
                        nc.vector.tensor_copy(yu[:], yqs[hh][0:HD, :])
                        yus.append(yu)

                def wo_load(Q, pc, eng=None):
                    # one batched 2MB DMA, default on the gpsimd queue:
                    # gates on AG(Q) completion and must not block the
                    # sync queue's xt loads for the next chunk.  The tail
                    # loads go on sync instead — gpsimd is blocked there
                    # by the AG(3) trigger until the collective finishes,
                    # while sync has nothing left to do.
                    eng = eng or nc.gpsimd
                    ytb = pc.tile([128, 16 * 512], bf16, tag="yt",
                                  bufs=2, name="yt")
                    eng.dma_start(
                        out=ytb[:].rearrange("p (g c) -> p g c", g=16),
                        in_=ago[Q][:].rearrange("(g p) c -> p g c",
                                                g=16))
                    return ytb

                def wo_chunk(Q, ytb, pc):
                    c0 = 512 * Q
                    for ob in range(2):
                        wp = ps_tile([128, 512], ("wp", "ms")[ob])
                        for ci in range(16):
                            nc.tensor.matmul(
                                wp[0:128, :],
                                wo_sb[ci][:, 128 * ob:128 * (ob + 1)],
                                ytb[:, 512 * ci:512 * (ci + 1)],
                                start=(ci == 0), stop=(ci == 15))
                        ot = pc.tile([128, 512], f32, tag="ot", bufs=3)
                        nc.scalar.copy(ot[:], wp[0:128, :])
                        nc.sync.dma_start(
                            out=outT[128 * ob:128 * (ob + 1),
                                     c0:c0 + 512],
                            in_=ot[:])

                # ================= fused chunk pipeline =================
                with tc.tile_pool(name="pa", bufs=1) as pa:

                    for j in range(NB):
                        nc.gpsimd.tensor_copy(vgs[j][:, HD:HD + 1],
                                              ones1[:])

                    yt_pref = {}
                    for Q in range(NSB):
                        c0 = 512 * Q
                        vt = qkv_chunk(Q, pa)
                        if Q == 3:
                            # lag-3: first consumer of gathered y sits a
                            # full 3 chunks behind AG(0)'s trigger, so core
                            # launch skew rarely stalls the PE here
                            wo_chunk(0, yt_pref.pop(0), pa)
                        yus = []
                        rdens = []
                        attn_pair(0, Q, rdens, yus)
                        if Q == 3:
                            wo_chunk(1, yt_pref.pop(1), pa)
                        attn_pair(1, Q, rdens, yus)
                        ytn = pbc.tile([64, NH_LOC * 512], bf16,
                                       tag="ytn", bufs=2, name="ytn")
                        for h in range(NH_LOC):
                            rb = pbc.tile([64, 512], f32, tag="rb", bufs=2)
                            nc.gpsimd.partition_broadcast(
                                rb[:], rdens[h][:])
                            nc.vector.tensor_mul(
                                ytn[:, 512 * h:512 * (h + 1)],
                                yus[h][:], rb[:])
                        # one batched agi write on the scalar queue (idle
                        # post-exp; keeps sync free for the next xt load)
                        nc.scalar.dma_start(
                            out=agi[Q][:].rearrange("(g p) c -> p g c",
                                                    g=NH_LOC),
                            in_=ytn[:].rearrange("p (g c) -> p g c",
                                                 g=NH_LOC))
                        nc.gpsimd.collective_compute(
                            "AllGather", mybir.AluOpType.bypass,
                            replica_groups=[list(range(N_CORES))],
                            ins=[agi[Q][:]], outs=[ago[Q][:]])
                        if Q == 2:
                            yt_pref[0] = wo_load(0, pa)
                            yt_pref[1] = wo_load(1, pa)
                        if Q == 3:
                            yt_pref[2] = wo_load(2, pa, eng=nc.sync)
                    wo_chunk(2, yt_pref.pop(2), pa)
                    yt_pref[3] = wo_load(3, pa, eng=nc.sync)
                    wo_chunk(3, yt_pref.pop(3), pa)

    nc.compile()
    return nc


_PERM = np.concatenate([np.arange(0, HD, 2), np.arange(1, HD, 2)])

# gathered-y row order is h-major: row 512h + 64c + d holds global channel
# 256c + 64h + d; permute wo's input dims to match
_CI_PERM = np.empty(C, np.int64)
for _h in range(NH_LOC):
    for _c in range(N_CORES):
        _CI_PERM[512 * _h + 64 * _c:512 * _h + 64 * _c + 64] = \
            np.arange(256 * _c + 64 * _h, 256 * _c + 64 * _h + 64)


def _prep_inputs(x, freqs_cos, freqs_sin, wq, wk, wv, wo):
    import ml_dtypes
    bf = ml_dtypes.bfloat16
    x = np.asarray(x, np.float32)
    wq = np.asarray(wq, np.float32)
    wk = np.asarray(wk, np.float32)
    wv = np.asarray(wv, np.float32)
    wo = np.asarray(wo, np.float32)
    fc = np.asarray(freqs_cos, np.float32).T   # [32, T]
    fs = np.asarray(freqs_sin, np.float32).T

    xT = np.ascontiguousarray(x[0].T).astype(bf)          # [C, T]
    cs128 = np.ascontiguousarray(np.concatenate([fc, fc, fc, fc], axis=0))
    sn128 = np.ascontiguousarray(np.concatenate([-fs, fs, -fs, fs], axis=0))

    in_maps = []
    for c in range(N_CORES):
        wq_c = wq[QD * c:QD * (c + 1), :].reshape(NH_LOC, HD, C)
        wq_c = wq_c[:, _PERM, :].reshape(QD, C)
        wk_c = wk[HD * c:HD * (c + 1), :][_PERM, :]
        wv_c = wv[HD * c:HD * (c + 1), :]
        in_maps.append({
            "xT": xT,
            "wqT": np.ascontiguousarray(wq_c.T).astype(bf),
            "wkvT": np.ascontiguousarray(
                np.concatenate([wk_c.T, wv_c.T], axis=1)).astype(bf),
            "woT": np.ascontiguousarray(
                wo[QD * c:QD * (c + 1), :].T).astype(bf),
            "cs128": cs128,
            "sn128": sn128,
        })
    return in_maps


def get_nc():
    if "nc" not in _CACHE:
        _CACHE["nc"] = _build()
    return _CACHE["nc"]


def kernel(x, freqs_cos, freqs_sin, wq, wk, wv, wo, **run_kwargs):
    from concourse.bass_utils import run_bass_kernel_spmd
    nc = get_nc()
    in_maps = _prep_inputs(x, freqs_cos, freqs_sin, wq, wk, wv, wo)
    res = run_bass_kernel_spmd(nc, in_maps, list(range(N_CORES)), **run_kwargs)
    outT = np.concatenate([res.results[c]["outT"] for c in range(N_CORES)],
                          axis=0)
    out = np.ascontiguousarray(outT.T).reshape(1, T, C).astype(np.float32)
    if run_kwargs:
        kernel.last_results = res
    return out
